# revision 84
# baseline (speedup 1.0000x reference)
"""Trainium2 Bass kernel for causal GQA self-attention (B=2,S=2048,D=1024,H=16,HKV=4,HD=64).

Sharding: 8 cores = DP(2 over batch) x TP(4 over GQA groups).
Each core computes, for one batch element and one GQA group (4 q heads + 1 kv head),
the partial output  y_group @ Wo[:, group_cols].T  (row-sharded Wo).
Host sums the 4 TP partials per batch element.

v2 design notes (vs v1 baseline at 251us NEFF):
- causal mask folded into the scores PSUM via an identity-matmul bias write
  (-50 on masked lanes) instead of a DVE multiply after exp
- pv matmuls fused per head-pair ([65,512] accumulators, 2 MMs/j instead of 4)
- normalize: reciprocal on the [1,512] denominator row + PE broadcast,
  instead of full [64,256] reciprocals per head
- rms factors via Copy/DVE-recip/Sqrt (single ACT table set in phase 1,
  Exp only in phase 2 -> 2 table loads instead of 9)
- v transposed via DMA xbar transpose (frees PE + DVE)
- phase 3 (Wo) interleaved into the attention block loop, one block delayed
- fp16 output (halves store traffic)
- per-chunk pipelined phase 1 emission
"""

import sys
from contextlib import ExitStack

sys.path.insert(0, "/opt/trn_rl_repo")

import numpy as np
import ml_dtypes

import concourse.bass as bass
import concourse.bacc as bacc
import concourse.tile as tile
import concourse.mybir as mybir
from concourse.bass_utils import run_bass_kernel_spmd

BF16 = mybir.dt.bfloat16
F32 = mybir.dt.float32
F16 = mybir.dt.float16
F8E4 = mybir.dt.float8e4
AF = mybir.ActivationFunctionType
BF16NP = ml_dtypes.bfloat16

import os
KDEBUG = int(os.environ.get("KDEBUG", "0"))

D, H, HKV, HD, B, S = 1024, 16, 4, 64, 2, 2048
HG = 4              # q heads per core
KV_DIM = HKV * HD   # 256
E = HG * HD         # 256 local q-proj dim
ROPE_BASE = 10000.0
EPS = float(np.finfo(np.float32).eps)
MASK_NEG = -50.0

NK = D // 128       # 8 contraction tiles for qkv projections
SQB = 256           # sq block size in attention
NB = S // SQB       # 8 blocks
NJ = S // 128       # 16 sk tiles
NC = S // 512       # 4 chunks of 512 in phase 1


def _consts():
    """Constant tensors baked into the NEFF (same for every core)."""
    i = np.arange(32, dtype=np.float64)
    inv_freq = 1.0 / (ROPE_BASE ** (2.0 * i / HD))
    pos = np.arange(S, dtype=np.float64)
    fr = pos[:, None] * inv_freq[None, :]           # [S, 32]
    cosT = np.cos(fr).T.astype(BF16NP)              # [32, S]
    sinT = np.sin(fr).T.astype(BF16NP)

    # mask bias for diagonal sk-tiles: pattern p in {0,1}
    # valid iff c >= 128*p + r   (r: sk row 0..127, c: sq col 0..255)
    r = np.arange(128)[:, None]
    c = np.arange(SQB)[None, :]
    mbs = []
    for p in range(2):
        m = np.where(c >= 128 * p + r, 0.0, MASK_NEG).astype(BF16NP)  # [128, 256]
        mbs.append(np.tile(m, (1, 2)))               # [128, 512] (2 head slots)

    sel36 = np.zeros((128, 36), dtype=BF16NP)        # q sumsq head selector
    for h in range(4):
        sel36[32 * h:32 * h + 32, h] = 1.0
    id128 = np.eye(128, dtype=BF16NP)
    return cosT, sinT, mbs, sel36, id128


def _build():
    nc = bacc.Bacc("TRN2", debug=False)

    xT_d = nc.dram_tensor("xT", [D, S], BF16, kind="ExternalInput")
    wq_d = nc.dram_tensor("wq", [128, NK, E], BF16, kind="ExternalInput")
    wkv_d = nc.dram_tensor("wkv", [128, NK, 128], BF16, kind="ExternalInput")
    wo_d = nc.dram_tensor("wo", [128, 2, D], BF16, kind="ExternalInput")
    gsel_d = nc.dram_tensor("gsel", [4, 128], BF16, kind="ExternalInput")
    out_d = nc.dram_tensor("out", [S, D], F16, kind="ExternalOutput")
    dbg = {}
    if KDEBUG:
        for nm, shp in [("d_qsb0", [128, S]), ("d_qsb1", [128, S]),
                        ("d_kvsb", [128, S]), ("d_qstd0", [128, S]),
                        ("d_qstd1", [128, S]), ("d_kdup", [128, S]),
                        ("d_vsb", [128, NJ, 66]), ("d_yn0", [128, S]),
                        ("d_yn1", [128, S]), ("d_pt", [128, 1024]),
                        ("d_pbs", [128, 512]), ("d_ft", [33, 512])]:
            dbg[nm] = nc.dram_tensor(nm, shp, BF16, kind="ExternalOutput")
        dbg["d_den"] = nc.dram_tensor("d_den", [2, 512], F32, kind="ExternalOutput")
        dbg["d_rbv"] = nc.dram_tensor("d_rbv", [2, 512], F32, kind="ExternalOutput")
        dbg["d_yt01"] = nc.dram_tensor("d_yt01", [64, 512], F32, kind="ExternalOutput")
        dbg["d_yt23"] = nc.dram_tensor("d_yt23", [64, 512], F32, kind="ExternalOutput")

    cosT, sinT, mbs, sel36, id128 = _consts()
    cs_d = nc.inline_tensor(np.concatenate([cosT, sinT], axis=1), "cs")  # [32,2S]
    mb_d = nc.inline_tensor(np.concatenate(mbs, axis=1), "mb")           # [128,2048]
    sel36_d = nc.inline_tensor(sel36, "sel36")
    id128_d = nc.inline_tensor(id128, "id128")

    with tile.TileContext(nc) as tc, ExitStack() as ctx:
        sp = ctx.enter_context(tc.tile_pool(name="static", bufs=1))

        def stile(shape, dt, tag):
            return sp.tile(shape, dt, name=tag, tag=tag)

        # ---- static SBUF tensors ----
        xt_all = stile([128, NK, S], BF16, "xt")
        xt = [xt_all[:, k, :] for k in range(NK)]
        wq = stile([128, NK, E], BF16, "wq")
        wkv = stile([128, NK, 128], BF16, "wkv")
        wo = stile([128, 2, D], BF16, "wo")
        cs = stile([128, 2 * S], BF16, "cs")          # [cos | sin]
        mbt = stile([128, 1024], BF16, "mbt")         # [maskbias p0 | p1]
        sel36_s = stile([128, 36], BF16, "sel36")
        id128_s = stile([128, 128], BF16, "id128")
        gsel_s = stile([4, 128], BF16, "gsel")
        onesr = stile([128, 64], BF16, "onesr")      # bf16 ones (k sumsq lhsT)
        onesf16 = stile([128, 64], F16, "onesf16")   # f16 ones (denom bcast lhsT)
        e8b = stile([128, 1], F32, "e8b")            # exp bias (0; kept as AP)

        qsb = [stile([128, S], BF16, f"qsb{m}") for m in range(2)]   # T/B packed
        kvsb = stile([128, S], BF16, "kvsb")          # k(0:64) | v(64:128)
        kb0 = stile([32, S], BF16, "kb0")             # k bottom half at partition 0
        # pair-packed q: rows 0:64 = even pair (h0,h2) hd dims, rows 64:128 =
        # odd pair (h1,h3); per block the cols are [hA 256 | hB 256]
        qp = stile([128, NB, 2, 256], BF16, "qp")
        kdup = stile([128, S], BF16, "kdup")          # [k ; k] for both row groups
        vsb = stile([128, NJ, 66], BF16, "vsb")       # [v(0:64) | ones(64) | pad]
        ynA = stile([128, 2, S], BF16, "yn")          # normalized y^T, both halves

        pbs = stile([128, 512], BF16, "pbs")          # bcast recip: rows0:64 p01, 64:128 p23

        # ---- const / weight loads (issue split across sync + scalar queues) ----
        # xt chunk 0 + weights first so projections can start ASAP
        nc.scalar.dma_start(wq[:], wq_d[:])
        for k in range(NK):
            eng = nc.sync if k % 2 == 0 else nc.scalar
            eng.dma_start(xt_all[:, k, 0:512], xT_d[128 * k:128 * (k + 1), 0:512])
        nc.sync.dma_start(wkv[:], wkv_d[:])
        nc.scalar.dma_start(cs[0:32, :], cs_d[:])
        nc.scalar.dma_start(cs[32:64, :], cs[0:32, :])
        nc.scalar.dma_start(cs[64:128, :], cs[0:64, :])
        nc.scalar.dma_start(mbt[:], mb_d[:])
        nc.scalar.dma_start(sel36_s[:], sel36_d[:])
        nc.scalar.dma_start(id128_s[:], id128_d[:])
        nc.scalar.dma_start(gsel_s[:], gsel_d[:])
        nc.scalar.dma_start(wo[:], wo_d[:])
        # remaining xt columns
        for k in range(NK):
            eng = nc.sync if k % 2 == 0 else nc.scalar
            eng.dma_start(xt_all[:, k, 512:S], xT_d[128 * k:128 * (k + 1), 512:S])
        nc.vector.memset(onesr[:], 1.0)
        nc.vector.memset(onesf16[:], 1.0)
        nc.vector.memset(e8b[:], 0.0)
        nc.vector.memset(vsb[:], 1.0)  # ones column at [:, j, 64]; 0:64 overwritten

        # ======== phase 1: projections + rms factors + rope (per 512-chunk) ==
        with (
            tc.tile_pool(name="pp", bufs=3, space=bass.MemorySpace.PSUM) as pp,
            tc.tile_pool(name="pper", bufs=1, space=bass.MemorySpace.PSUM) as pper,
            tc.tile_pool(name="pfb", bufs=1, space=bass.MemorySpace.PSUM) as pfb,
            tc.tile_pool(name="pfk", bufs=1, space=bass.MemorySpace.PSUM) as pfk,
            tc.tile_pool(name="lns", bufs=2) as lns,
            tc.tile_pool(name="rt", bufs=2) as rt,
        ):
            # PE warmup: ~8us of dummy matmul activity while input DMAs land,
            # so the HAM clock gate reaches K=8/8 (2.4 GHz) before real work.
            wtile = pper.tile([1, 64], F32, name="warm", tag="psqk")
            for i in range(80):
                nc.tensor.matmul(wtile[:], onesr[0:64, 0:1], onesr[0:64, :],
                                 start=(i == 0), stop=(i == 79),
                                 skip_group_check=True)
            def proj_chunk(n):
                sl = slice(512 * n, 512 * (n + 1))
                pq0 = pp.tile([128, 512], F32, name="pq", tag="pq")
                pq1 = pp.tile([128, 512], F32, name="pq", tag="pq")
                pkv = pp.tile([128, 512], F32, name="pq", tag="pq")
                for k in range(NK):
                    nc.tensor.matmul(pq0[:], wq[:, k, 0:128], xt[k][:, sl],
                                     start=(k == 0), stop=(k == NK - 1))
                for k in range(NK):
                    nc.tensor.matmul(pq1[:], wq[:, k, 128:256], xt[k][:, sl],
                                     start=(k == 0), stop=(k == NK - 1))
                for k in range(NK):
                    nc.tensor.matmul(pkv[:], wkv[:, k, :], xt[k][:, sl],
                                     start=(k == 0), stop=(k == NK - 1))
                # drains run on ACT/DVE right away; PE moves on to next chunk
                nc.vector.tensor_copy(qsb[0][:, sl], pq0[:])
                nc.scalar.copy(qsb[1][:, sl], pq1[:])
                nc.scalar.copy(kvsb[:, sl], pkv[:])
                nc.sync.dma_start(kb0[:, sl], kvsb[32:64, sl])

            def post_chunk(n):
                """Everything downstream of the chunk's projections; deferred
                one chunk so it never blocks the PE projection stream."""
                sl = slice(512 * n, 512 * (n + 1))
                slc = slice(512 * n, 512 * (n + 1))
                sls = slice(S + 512 * n, S + 512 * (n + 1))
                # v transpose via PE ([64,128] -> [128,64])
                for t in range(4):
                    st = 4 * n + t
                    ptr = pp.tile([128, 64], BF16, name="ptr", tag="ptr", bufs=2)
                    nc.tensor.transpose(
                        ptr[:], kvsb[64:128, 128 * st:128 * (st + 1)],
                        id128_s[64:128, 64:128])
                    if t % 2 == 0:
                        nc.scalar.copy(vsb[:, st, 0:64], ptr[:])
                    else:
                        nc.vector.tensor_copy(vsb[:, st, 0:64], ptr[:])
                # squared sums
                sq0 = rt.tile([128, 512], BF16, name="sq0", tag="sq0")
                sq1 = rt.tile([128, 512], BF16, name="sq1", tag="sq1")
                sqk = rt.tile([64, 512], BF16, name="sqk", tag="sqk")
                nc.vector.tensor_mul(sq0[:], qsb[0][:, sl], qsb[0][:, sl])
                nc.vector.tensor_mul(sq1[:], qsb[1][:, sl], qsb[1][:, sl])
                nc.vector.tensor_mul(sqk[:], kvsb[0:64, sl], kvsb[0:64, sl])
                # reduce to per-head sums (PE): rows 0:4 = q heads, row 32 = k
                psqk = pper.tile([36, 512], F32, name="psqk", tag="psqk")
                nc.tensor.matmul(psqk[:], sel36_s[:], sq0[:], start=True, stop=False)
                nc.tensor.matmul(psqk[:], sel36_s[:], sq1[:], start=False, stop=True)
                nc.tensor.matmul(psqk[32:33, :], onesr[0:64, 0:1], sqk[:],
                                 start=False, stop=True, skip_group_check=True)
                # f = sqrt(1 / (ssq/HD + eps)); all ACT funcs from one table set
                vt = lns.tile([33, 512], F32, name="vt", tag="vt")
                nc.scalar.activation(vt[:], psqk[0:33, :], AF.Copy,
                                     bias=EPS, scale=1.0 / HD)
                rc = lns.tile([33, 512], F32, name="rc", tag="rc")
                nc.vector.reciprocal_approx_fast(rc[:], vt[:])
                ft = lns.tile([33, 512], BF16, name="ft", tag="ft")
                nc.scalar.activation(ft[:], rc[:], AF.Sqrt)
                # broadcast factors along hd rows via PE (gain/8 folded in gsel)
                fbq_ps = pfb.tile([128, 512], F32, name="fbq", tag="fbq")
                nc.tensor.matmul(fbq_ps[:], gsel_s[:], ft[0:4, :], start=True, stop=True)
                fbk_ps = pfk.tile([64, 512], F32, name="fbk", tag="fbk")
                nc.tensor.matmul(fbk_ps[:], onesr[32:33, 0:64], ft[32:33, :],
                                 start=True, stop=True)
                fbq = lns.tile([128, 512], BF16, name="fbq_s", tag="fbq_s")
                fbk = lns.tile([64, 512], BF16, name="fbk_s", tag="fbk_s")
                nc.scalar.copy(fbq[:], fbq_ps[:])
                nc.scalar.copy(fbk[:], fbk_ps[:])
                # rope + scale (DVE)
                t1 = rt.tile([128, 512], BF16, name="t1", tag="t1")
                t2 = rt.tile([128, 512], BF16, name="t2", tag="t2")
                qr0 = rt.tile([128, 512], BF16, name="qr0", tag="qr0")
                qr1 = rt.tile([128, 512], BF16, name="qr1", tag="qr1")
                nc.vector.tensor_mul(t1[:], qsb[0][:, sl], cs[:, slc])
                nc.vector.tensor_mul(t2[:], qsb[1][:, sl], cs[:, sls])
                nc.vector.tensor_add(t1[:], t1[:], t2[:])
                nc.vector.tensor_mul(qr0[:], t1[:], fbq[:])
                u1 = rt.tile([128, 512], BF16, name="u1", tag="u1")
                u2 = rt.tile([128, 512], BF16, name="u2", tag="u2")
                nc.vector.tensor_mul(u1[:], qsb[1][:, sl], cs[:, slc])
                nc.vector.tensor_mul(u2[:], qsb[0][:, sl], cs[:, sls])
                nc.vector.tensor_sub(u1[:], u1[:], u2[:])
                nc.vector.tensor_mul(qr1[:], u1[:], fbq[:])
                k1 = rt.tile([32, 512], BF16, name="k1", tag="k1")
                k2 = rt.tile([32, 512], BF16, name="k2", tag="k2")
                kw0 = rt.tile([32, 512], BF16, name="kw0", tag="kw0")
                kw1 = rt.tile([32, 512], BF16, name="kw1", tag="kw1")
                nc.vector.tensor_mul(k1[:], kvsb[0:32, sl], cs[0:32, slc])
                nc.vector.tensor_mul(k2[:], kb0[:, sl], cs[0:32, sls])
                nc.vector.tensor_add(k1[:], k1[:], k2[:])
                nc.vector.tensor_mul(kw0[:], k1[:], fbk[0:32, :])
                k3 = rt.tile([32, 512], BF16, name="k3", tag="k3")
                k4 = rt.tile([32, 512], BF16, name="k4", tag="k4")
                nc.vector.tensor_mul(k3[:], kb0[:, sl], cs[0:32, slc])
                nc.vector.tensor_mul(k4[:], kvsb[0:32, sl], cs[0:32, sls])
                nc.vector.tensor_sub(k3[:], k3[:], k4[:])
                nc.vector.tensor_mul(kw1[:], k3[:], fbk[0:32, :])
                # reassemble pair-packed q layout (DMA partition moves)
                for h, (rb, slot) in ((0, (0, 0)), (2, (0, 1)),
                                      (1, (64, 0)), (3, (64, 1))):
                    hs = slice(32 * h, 32 * h + 32)
                    bsl = slice(2 * n, 2 * n + 2)
                    nc.sync.dma_start(
                        qp[rb:rb + 32, bsl, slot, :],
                        qr0[hs, :].rearrange("p (b c) -> p b c", b=2))
                    nc.sync.dma_start(
                        qp[rb + 32:rb + 64, bsl, slot, :],
                        qr1[hs, :].rearrange("p (b c) -> p b c", b=2))
                nc.sync.dma_start(kdup[0:32, sl], kw0[:])
                nc.sync.dma_start(kdup[32:64, sl], kw1[:])
                nc.sync.dma_start(kdup[64:96, sl], kw0[:])
                nc.sync.dma_start(kdup[96:128, sl], kw1[:])

            proj_chunk(0)
            for n in range(1, NC):
                proj_chunk(n)
                post_chunk(n - 1)
            post_chunk(NC - 1)

        # ======== phase 2+3: attention blocks with interleaved Wo ========
        with (
            tc.tile_pool(name="ps", bufs=2, space=bass.MemorySpace.PSUM) as ps,
            tc.tile_pool(name="py01", bufs=2, space=bass.MemorySpace.PSUM) as py01,
            tc.tile_pool(name="py23", bufs=2, space=bass.MemorySpace.PSUM) as py23,
            tc.tile_pool(name="pa", bufs=4) as pa,
            tc.tile_pool(name="sst", bufs=2) as sst,
            tc.tile_pool(name="ob", bufs=2) as ob,
        ):
            def emit_wo(b):
                """Output projection for sq block b (yn cols 256b:256b+256)."""
                for t2 in range(2):
                    sti = 2 * b + t2
                    ssl = slice(128 * sti, 128 * (sti + 1))
                    ot = ob.tile([128, D], F16, name="ot", tag="ot")
                    for half in range(2):
                        dsl = slice(512 * half, 512 * (half + 1))
                        pot = ps.tile([128, 512], F32, name="st", tag="st")
                        nc.tensor.matmul(pot[:], ynA[:, 0, ssl], wo[:, 0, dsl],
                                         start=True, stop=False)
                        nc.tensor.matmul(pot[:], ynA[:, 1, ssl], wo[:, 1, dsl],
                                         start=False, stop=True)
                        # drains split DVE/ACT to balance engine load
                        if half == 0:
                            nc.scalar.copy(ot[:, dsl], pot[:])
                        else:
                            nc.vector.tensor_copy(ot[:, dsl], pot[:])
                    nc.gpsimd.dma_start(out_d[ssl, :], ot[:])

            def finish_normalize(st8):
                """PE broadcast + multiplies for a block whose recip is ready."""
                b, yt01, yt23, rbc = st8
                sq = slice(SQB * b, SQB * (b + 1))
                pbb = ps.tile([128, 512], F32, name="st", tag="st")
                nc.tensor.matmul(pbb[0:64, :], onesf16[0:1, 0:64],
                                 rbc[0:1, 0:512],
                                 start=True, stop=True, skip_group_check=True)
                nc.tensor.matmul(pbb[64:128, :], onesf16[0:1, 0:64],
                                 rbc[0:1, 512:1024],
                                 start=True, stop=True, skip_group_check=True)
                nc.vector.tensor_copy(pbs[:], pbb[:])
                # h0 -> yn slot 0 rows 0:64, h2 -> slot 1 rows 0:64 (direct)
                nc.vector.tensor_mul(
                    ynA[0:64, :, sq],
                    yt01[0:64, :].rearrange("p (u c) -> p u c", u=2),
                    pbs[0:64, :].rearrange("p (u c) -> p u c", u=2))
                # h1/h3 -> yn rows 64:128 (via shift DMA)
                sg1 = sst.tile([64, 2, 256], BF16, name="sg1", tag="sg1")
                nc.vector.tensor_mul(
                    sg1[:],
                    yt23[0:64, :].rearrange("p (u c) -> p u c", u=2),
                    pbs[64:128, :].rearrange("p (u c) -> p u c", u=2))
                nc.sync.dma_start(ynA[64:128, :, sq], sg1[:])
                if KDEBUG and b == 0:
                    nc.sync.dma_start(dbg["d_pbs"][:], pbs[:])

            pend = None
            for b in range(NB):
                sq = slice(SQB * b, SQB * (b + 1))
                jmax = 2 * b + 1
                # inject the deferred (previous-block) normalize + Wo early in
                # this block's j-loop so the PE flows straight across the
                # block boundary
                inject_at = min(4, jmax)
                yt01 = py01.tile([65, 512], F32, name="yt01", tag="yt01")
                yt23 = py23.tile([65, 512], F32, name="yt23", tag="yt23")
                for j in range(jmax + 1):
                    if j == inject_at:
                        if pend is not None:
                            finish_normalize(pend)
                        if b >= 2:
                            emit_wo(b - 2)
                    jt = slice(128 * j, 128 * (j + 1))
                    stl = ps.tile([128, 1024], F32, name="st", tag="st")
                    diag = j - 2 * b
                    if diag >= 0:
                        mbsl = slice(512 * diag, 512 * (diag + 1))
                        nc.tensor.matmul(stl[:, 0:512], id128_s[:], mbt[:, mbsl],
                                         start=True, stop=False,
                                         skip_group_check=True)
                        nc.tensor.matmul(stl[:, 512:1024], id128_s[:], mbt[:, mbsl],
                                         start=True, stop=False,
                                         skip_group_check=True)
                    sflag = diag < 0
                    # cols: h0 0:256 | h2 256:512 | h1 512:768 | h3 768:1024
                    nc.tensor.matmul(stl[:, 0:512], kdup[0:64, jt],
                                     qp[0:64, b, :, :], start=sflag, stop=True,
                                     skip_group_check=True)
                    nc.tensor.matmul(stl[:, 512:1024], kdup[64:128, jt],
                                     qp[64:128, b, :, :], start=sflag, stop=True,
                                     skip_group_check=True)
                    pt = pa.tile([128, 1024], BF16, name="pt", tag="pt")
                    nc.scalar.activation(pt[:], stl[:], AF.Exp, bias=e8b[:, :])
                    if KDEBUG and b == 0 and j == 0:
                        nc.sync.dma_start(dbg["d_pt"][:], pt[:])
                    # pv: pair01 = (h0,h2) cols 0:512, pair23 = (h1,h3) 512:1024
                    nc.tensor.matmul(yt01[:], vsb[:, j, 0:65], pt[:, 0:512],
                                     start=(j == 0), stop=(j == jmax),
                                     skip_group_check=True)
                    nc.tensor.matmul(yt23[:], vsb[:, j, 0:65], pt[:, 512:1024],
                                     start=(j == 0), stop=(j == jmax),
                                     skip_group_check=True)
                # normalize, DVE/DMA part; denominators at psum row 64.
                # custom-DVE ops misbehave on HW at partition base 64, so:
                # tracked DVE copy (psum row 64 -> sbuf row 64), DMA shift to
                # partition 0, recip at base 0 (known-good), tracked f16 copy
                # (DVE order covers the untracked recip write).  The PE-side
                # broadcast + multiplies are deferred one block so the PE
                # queue never waits on this chain.
                dcp = sst.tile([65, 1024], F32, name="dcp", tag="dcp")
                nc.vector.tensor_copy(dcp[64:65, 0:512], yt01[64:65, :])
                nc.vector.tensor_copy(dcp[64:65, 512:1024], yt23[64:65, :])
                dlow = sst.tile([1, 1024], F32, name="dlow", tag="dlow")
                nc.sync.dma_start(dlow[0:1, :], dcp[64:65, 0:1024])
                rlow = sst.tile([1, 1024], F32, name="rlow", tag="rlow")
                nc.vector.reciprocal_approx_fast(rlow[0:1, :], dlow[0:1, :])
                rbc = sst.tile([1, 1024], F16, name="rbc", tag="rbc")
                nc.vector.tensor_copy(rbc[0:1, :], rlow[0:1, :])
                if KDEBUG and b == 0:
                    dstage = sst.tile([65, 512], F32, name="dst", tag="dst")
                    dstage2 = sst.tile([65, 512], F32, name="dst2", tag="dst2")
                    nc.vector.tensor_copy(dstage[64:65, :], yt01[64:65, :])
                    nc.vector.tensor_copy(dstage2[64:65, :], yt23[64:65, :])
                    nc.sync.dma_start(dbg["d_den"][0:1, :], dstage[64:65, :])
                    nc.sync.dma_start(dbg["d_den"][1:2, :], dstage2[64:65, :])
                    nc.vector.tensor_copy(dstage[0:64, :], yt01[0:64, :])
                    nc.vector.tensor_copy(dstage2[0:64, :], yt23[0:64, :])
                    nc.sync.dma_start(dbg["d_yt01"][:], dstage[0:64, :])
                    nc.sync.dma_start(dbg["d_yt23"][:], dstage2[0:64, :])
                    nc.sync.dma_start(dbg["d_rbv"][0:1, :], rlow[0:1, 0:512])
                    nc.sync.dma_start(dbg["d_rbv"][1:2, :], rlow[0:1, 512:1024])
                pend = (b, yt01, yt23, rbc)
            finish_normalize(pend)
            emit_wo(NB - 2)
            emit_wo(NB - 1)
            if KDEBUG:
                nc.sync.dma_start(dbg["d_qsb0"][:], qsb[0][:])
                nc.sync.dma_start(dbg["d_qsb1"][:], qsb[1][:])
                nc.sync.dma_start(dbg["d_kvsb"][:], kvsb[:])
                nc.sync.dma_start(dbg["d_kdup"][:], kdup[:])
                nc.sync.dma_start(dbg["d_vsb"][:], vsb[:])
                nc.sync.dma_start(dbg["d_yn0"][:], ynA[:, 0, :])
                nc.sync.dma_start(dbg["d_yn1"][:], ynA[:, 1, :])

    nc.finalize()
    return nc


_NC = None


def _get_nc():
    global _NC
    if _NC is None:
        _NC = _build()
    return _NC


def _perm():
    tops = [h * 64 + i for h in range(HG) for i in range(32)]
    bots = [h * 64 + 32 + i for h in range(HG) for i in range(32)]
    return tops + bots


def build_inmaps(x, Wq, Wk, Wv, Wo, q_gain):
    x = np.asarray(x, dtype=np.float32)
    Wq = np.asarray(Wq, dtype=np.float32)
    Wk = np.asarray(Wk, dtype=np.float32)
    Wv = np.asarray(Wv, dtype=np.float32)
    Wo = np.asarray(Wo, dtype=np.float32)
    q_gain = np.asarray(q_gain, dtype=np.float32)

    perm = _perm()
    xTs = [np.ascontiguousarray(x[dp].T).astype(BF16NP) for dp in range(2)]
    tp_maps = []
    for tp in range(4):
        wq_sel = Wq[tp * E:(tp + 1) * E].T[:, perm]          # [D, 256] permuted
        wq_t = np.ascontiguousarray(
            wq_sel.astype(BF16NP).reshape(NK, 128, E).transpose(1, 0, 2))
        wk_sel = Wk[tp * HD:(tp + 1) * HD].T                  # [D, 64]
        wv_sel = Wv[tp * HD:(tp + 1) * HD].T
        wkv_t = np.concatenate([wk_sel, wv_sel], axis=1).astype(BF16NP)
        wkv_t = np.ascontiguousarray(
            wkv_t.reshape(NK, 128, 128).transpose(1, 0, 2))
        wo_sel = Wo[:, tp * E:(tp + 1) * E].T                 # [256, D]
        wo_t = np.ascontiguousarray(
            wo_sel.astype(BF16NP).reshape(2, 128, D).transpose(1, 0, 2))
        g = q_gain[tp * HG:(tp + 1) * HG].astype(np.float32)
        gsel = np.zeros((4, 128), dtype=BF16NP)
        for h in range(4):
            gsel[h, 32 * h:32 * h + 32] = BF16NP(g[h] / 8.0)
        tp_maps.append({"wq": wq_t, "wkv": wkv_t, "wo": wo_t, "gsel": gsel})
    in_maps = []
    for c in range(8):
        dp, tp = divmod(c, 4)
        m = dict(tp_maps[tp])
        m["xT"] = xTs[dp]
        in_maps.append(m)
    return in_maps


def kernel(x, Wq, Wk, Wv, Wo, q_gain):
    in_maps = build_inmaps(x, Wq, Wk, Wv, Wo, q_gain)
    nc = _get_nc()
    res = run_bass_kernel_spmd(nc, in_maps, core_ids=list(range(8)))
    out = np.zeros((B, S, D), dtype=np.float32)
    for c in range(8):
        out[c // 4] += res.results[c]["out"].astype(np.float32)
    return out


# revision 85
# speedup vs baseline: 1.1220x; 1.1220x over previous
"""Trainium2 Bass kernel for causal GQA self-attention (B=2,S=2048,D=1024,H=16,HKV=4,HD=64).

Sharding: 8 cores = DP(2 over batch) x TP(4 over GQA groups).
Each core computes, for one batch element and one GQA group (4 q heads + 1 kv head),
the partial output  y_group @ Wo[:, group_cols].T  (row-sharded Wo).
Host sums the 4 TP partials per batch element.

v2 design notes (vs v1 baseline at 251us NEFF):
- causal mask folded into the scores PSUM via an identity-matmul bias write
  (-50 on masked lanes) instead of a DVE multiply after exp
- pv matmuls fused per head-pair ([65,512] accumulators, 2 MMs/j instead of 4)
- normalize: reciprocal on the [1,512] denominator row + PE broadcast,
  instead of full [64,256] reciprocals per head
- rms factors via Copy/DVE-recip/Sqrt (single ACT table set in phase 1,
  Exp only in phase 2 -> 2 table loads instead of 9)
- v transposed via DMA xbar transpose (frees PE + DVE)
- phase 3 (Wo) interleaved into the attention block loop, one block delayed
- fp16 output (halves store traffic)
- per-chunk pipelined phase 1 emission
"""

import sys
from contextlib import ExitStack

sys.path.insert(0, "/opt/trn_rl_repo")

import numpy as np
import ml_dtypes

import concourse.bass as bass
import concourse.bacc as bacc
import concourse.tile as tile
import concourse.mybir as mybir
from concourse.bass_utils import run_bass_kernel_spmd

BF16 = mybir.dt.bfloat16
F32 = mybir.dt.float32
F16 = mybir.dt.float16
F8E4 = mybir.dt.float8e4
AF = mybir.ActivationFunctionType
BF16NP = ml_dtypes.bfloat16

import os
KDEBUG = int(os.environ.get("KDEBUG", "0"))

D, H, HKV, HD, B, S = 1024, 16, 4, 64, 2, 2048
HG = 4              # q heads per core
KV_DIM = HKV * HD   # 256
E = HG * HD         # 256 local q-proj dim
ROPE_BASE = 10000.0
EPS = float(np.finfo(np.float32).eps)
MASK_NEG = -50.0

NK = D // 128       # 8 contraction tiles for qkv projections
SQB = 256           # sq block size in attention
NB = S // SQB       # 8 blocks
NJ = S // 128       # 16 sk tiles
NC = S // 512       # 4 chunks of 512 in phase 1


def _consts():
    """Constant tensors baked into the NEFF (same for every core)."""
    i = np.arange(32, dtype=np.float64)
    inv_freq = 1.0 / (ROPE_BASE ** (2.0 * i / HD))
    pos = np.arange(S, dtype=np.float64)
    fr = pos[:, None] * inv_freq[None, :]           # [S, 32]
    cosT = np.cos(fr).T.astype(BF16NP)              # [32, S]
    sinT = np.sin(fr).T.astype(BF16NP)

    # mask bias for diagonal sk-tiles: pattern p in {0,1}
    # valid iff c >= 128*p + r   (r: sk row 0..127, c: sq col 0..255)
    r = np.arange(128)[:, None]
    c = np.arange(SQB)[None, :]
    mbs = []
    for p in range(2):
        m = np.where(c >= 128 * p + r, 0.0, MASK_NEG).astype(BF16NP)  # [128, 256]
        mbs.append(np.tile(m, (1, 2)))               # [128, 512] (2 head slots)

    sel36 = np.zeros((128, 36), dtype=BF16NP)        # q sumsq head selector
    for h in range(4):
        sel36[32 * h:32 * h + 32, h] = 1.0
    id128 = np.eye(128, dtype=BF16NP)
    return cosT, sinT, mbs, sel36, id128


def _build():
    nc = bacc.Bacc("TRN2", debug=False)

    xT_d = nc.dram_tensor("xT", [D, S], BF16, kind="ExternalInput")
    wq_d = nc.dram_tensor("wq", [128, NK, E], BF16, kind="ExternalInput")
    wkv_d = nc.dram_tensor("wkv", [128, NK, 128], BF16, kind="ExternalInput")
    wo_d = nc.dram_tensor("wo", [128, 2, D], BF16, kind="ExternalInput")
    gsel_d = nc.dram_tensor("gsel", [4, 128], BF16, kind="ExternalInput")
    out_d = nc.dram_tensor("out", [S, D], F16, kind="ExternalOutput")
    dbg = {}
    if KDEBUG:
        for nm, shp in [("d_qsb0", [128, S]), ("d_qsb1", [128, S]),
                        ("d_kvsb", [128, S]), ("d_qstd0", [128, S]),
                        ("d_qstd1", [128, S]), ("d_kdup", [128, S]),
                        ("d_vsb", [128, NJ, 66]), ("d_yn0", [128, S]),
                        ("d_yn1", [128, S]), ("d_pt", [128, 1024]),
                        ("d_pbs", [128, 512]), ("d_ft", [33, 512])]:
            dbg[nm] = nc.dram_tensor(nm, shp, BF16, kind="ExternalOutput")
        dbg["d_den"] = nc.dram_tensor("d_den", [2, 512], F32, kind="ExternalOutput")
        dbg["d_rbv"] = nc.dram_tensor("d_rbv", [2, 512], F32, kind="ExternalOutput")
        dbg["d_yt01"] = nc.dram_tensor("d_yt01", [64, 512], F32, kind="ExternalOutput")
        dbg["d_yt23"] = nc.dram_tensor("d_yt23", [64, 512], F32, kind="ExternalOutput")

    cosT, sinT, mbs, sel36, id128 = _consts()
    cs_d = nc.inline_tensor(np.concatenate([cosT, sinT], axis=1), "cs")  # [32,2S]
    mb_d = nc.inline_tensor(np.concatenate(mbs, axis=1), "mb")           # [128,2048]
    sel36_d = nc.inline_tensor(sel36, "sel36")
    id128_d = nc.inline_tensor(id128, "id128")

    with tile.TileContext(nc) as tc, ExitStack() as ctx:
        sp = ctx.enter_context(tc.tile_pool(name="static", bufs=1))

        def stile(shape, dt, tag):
            return sp.tile(shape, dt, name=tag, tag=tag)

        # ---- static SBUF tensors ----
        xt_all = stile([128, NK, S], BF16, "xt")
        xt = [xt_all[:, k, :] for k in range(NK)]
        wq = stile([128, NK, E], BF16, "wq")
        wkv = stile([128, NK, 128], BF16, "wkv")
        wo = stile([128, 2, D], BF16, "wo")
        cs = stile([128, 2 * S], BF16, "cs")          # [cos | sin]
        mbt = stile([128, 1024], BF16, "mbt")         # [maskbias p0 | p1]
        sel36_s = stile([128, 36], BF16, "sel36")
        id128_s = stile([128, 128], BF16, "id128")
        gsel_s = stile([4, 128], BF16, "gsel")
        onesr = stile([128, 64], BF16, "onesr")      # bf16 ones (k sumsq lhsT)
        onesf16 = stile([128, 64], F16, "onesf16")   # f16 ones (denom bcast lhsT)
        e8b = stile([128, 1], F32, "e8b")            # exp bias (0; kept as AP)

        qsb = [stile([128, S], BF16, f"qsb{m}") for m in range(2)]   # T/B packed
        kvsb = stile([128, S], BF16, "kvsb")          # k(0:64) | v(64:128)
        kb0 = stile([32, S], BF16, "kb0")             # k bottom half at partition 0
        # pair-packed q: rows 0:64 = even pair (h0,h2) hd dims, rows 64:128 =
        # odd pair (h1,h3); per block the cols are [hA 256 | hB 256]
        qp = stile([128, NB, 2, 256], BF16, "qp")
        kdup = stile([128, S], BF16, "kdup")          # [k ; k] for both row groups
        vsb = stile([128, NJ, 66], BF16, "vsb")       # [v(0:64) | ones(64) | pad]
        ynA = stile([128, 2, S], BF16, "yn")          # normalized y^T, both halves

        pbs = stile([128, 512], BF16, "pbs")          # bcast recip: rows0:64 p01, 64:128 p23

        # ---- const / weight loads (issue split across sync + scalar queues) ----
        # xt chunk 0 + weights first so projections can start ASAP
        nc.scalar.dma_start(wq[:], wq_d[:])
        for k in range(NK):
            eng = nc.sync if k % 2 == 0 else nc.scalar
            eng.dma_start(xt_all[:, k, 0:512], xT_d[128 * k:128 * (k + 1), 0:512])
        nc.sync.dma_start(wkv[:], wkv_d[:])
        nc.scalar.dma_start(cs[0:32, :], cs_d[:])
        nc.scalar.dma_start(cs[32:64, :], cs[0:32, :])
        nc.scalar.dma_start(cs[64:128, :], cs[0:64, :])
        nc.scalar.dma_start(mbt[:], mb_d[:])
        nc.scalar.dma_start(sel36_s[:], sel36_d[:])
        nc.scalar.dma_start(id128_s[:], id128_d[:])
        nc.scalar.dma_start(gsel_s[:], gsel_d[:])
        nc.scalar.dma_start(wo[:], wo_d[:])
        # remaining xt columns
        for k in range(NK):
            eng = nc.sync if k % 2 == 0 else nc.scalar
            eng.dma_start(xt_all[:, k, 512:S], xT_d[128 * k:128 * (k + 1), 512:S])
        nc.vector.memset(onesr[:], 1.0)
        nc.vector.memset(onesf16[:], 1.0)
        nc.vector.memset(e8b[:], 0.0)
        nc.vector.memset(vsb[:], 1.0)  # ones column at [:, j, 64]; 0:64 overwritten

        # ======== phase 1: projections + rms factors + rope (per 512-chunk) ==
        with (
            tc.tile_pool(name="pp", bufs=3, space=bass.MemorySpace.PSUM) as pp,
            tc.tile_pool(name="pper", bufs=1, space=bass.MemorySpace.PSUM) as pper,
            tc.tile_pool(name="pfb", bufs=1, space=bass.MemorySpace.PSUM) as pfb,
            tc.tile_pool(name="pfk", bufs=1, space=bass.MemorySpace.PSUM) as pfk,
            tc.tile_pool(name="lns", bufs=2) as lns,
            tc.tile_pool(name="rt", bufs=2) as rt,
        ):
            # PE warmup: ~8us of dummy matmul activity while input DMAs land,
            # so the HAM clock gate reaches K=8/8 (2.4 GHz) before real work.
            wtile = pper.tile([1, 64], F32, name="warm", tag="psqk")
            for i in range(80):
                nc.tensor.matmul(wtile[:], onesr[0:64, 0:1], onesr[0:64, :],
                                 start=(i == 0), stop=(i == 79),
                                 skip_group_check=True)
            def proj_chunk(n):
                sl = slice(512 * n, 512 * (n + 1))
                pq0 = pp.tile([128, 512], F32, name="pq", tag="pq")
                pq1 = pp.tile([128, 512], F32, name="pq", tag="pq")
                pkv = pp.tile([128, 512], F32, name="pq", tag="pq")
                for k in range(NK):
                    nc.tensor.matmul(pq0[:], wq[:, k, 0:128], xt[k][:, sl],
                                     start=(k == 0), stop=(k == NK - 1))
                for k in range(NK):
                    nc.tensor.matmul(pq1[:], wq[:, k, 128:256], xt[k][:, sl],
                                     start=(k == 0), stop=(k == NK - 1))
                for k in range(NK):
                    nc.tensor.matmul(pkv[:], wkv[:, k, :], xt[k][:, sl],
                                     start=(k == 0), stop=(k == NK - 1))
                # drains run on ACT/DVE right away; PE moves on to next chunk
                nc.vector.tensor_copy(qsb[0][:, sl], pq0[:])
                nc.scalar.copy(qsb[1][:, sl], pq1[:])
                nc.scalar.copy(kvsb[:, sl], pkv[:])
                nc.sync.dma_start(kb0[:, sl], kvsb[32:64, sl])

            def post_chunk(n):
                """Everything downstream of the chunk's projections; deferred
                one chunk so it never blocks the PE projection stream."""
                sl = slice(512 * n, 512 * (n + 1))
                slc = slice(512 * n, 512 * (n + 1))
                sls = slice(S + 512 * n, S + 512 * (n + 1))
                # v transpose via PE ([64,128] -> [128,64])
                for t in range(4):
                    st = 4 * n + t
                    ptr = pp.tile([128, 64], BF16, name="ptr", tag="ptr", bufs=2)
                    nc.tensor.transpose(
                        ptr[:], kvsb[64:128, 128 * st:128 * (st + 1)],
                        id128_s[64:128, 64:128])
                    if t % 2 == 0:
                        nc.scalar.copy(vsb[:, st, 0:64], ptr[:])
                    else:
                        nc.vector.tensor_copy(vsb[:, st, 0:64], ptr[:])
                # squared sums
                sq0 = rt.tile([128, 512], BF16, name="sq0", tag="sq0")
                sq1 = rt.tile([128, 512], BF16, name="sq1", tag="sq1")
                sqk = rt.tile([64, 512], BF16, name="sqk", tag="sqk")
                nc.vector.tensor_mul(sq0[:], qsb[0][:, sl], qsb[0][:, sl])
                nc.vector.tensor_mul(sq1[:], qsb[1][:, sl], qsb[1][:, sl])
                nc.vector.tensor_mul(sqk[:], kvsb[0:64, sl], kvsb[0:64, sl])
                # reduce to per-head sums (PE): rows 0:4 = q heads, row 32 = k
                psqk = pper.tile([36, 512], F32, name="psqk", tag="psqk")
                nc.tensor.matmul(psqk[:], sel36_s[:], sq0[:], start=True, stop=False)
                nc.tensor.matmul(psqk[:], sel36_s[:], sq1[:], start=False, stop=True)
                nc.tensor.matmul(psqk[32:33, :], onesr[0:64, 0:1], sqk[:],
                                 start=False, stop=True, skip_group_check=True)
                # f = sqrt(1 / (ssq/HD + eps)); all ACT funcs from one table set
                vt = lns.tile([33, 512], F32, name="vt", tag="vt")
                nc.scalar.activation(vt[:], psqk[0:33, :], AF.Copy,
                                     bias=EPS, scale=1.0 / HD)
                rc = lns.tile([33, 512], F32, name="rc", tag="rc")
                nc.vector.reciprocal_approx_fast(rc[:], vt[:])
                ft = lns.tile([33, 512], BF16, name="ft", tag="ft")
                nc.scalar.activation(ft[:], rc[:], AF.Sqrt)
                # broadcast factors along hd rows via PE (gain/8 folded in gsel)
                fbq_ps = pfb.tile([128, 512], F32, name="fbq", tag="fbq")
                nc.tensor.matmul(fbq_ps[:], gsel_s[:], ft[0:4, :], start=True, stop=True)
                fbk_ps = pfk.tile([64, 512], F32, name="fbk", tag="fbk")
                nc.tensor.matmul(fbk_ps[:], onesr[32:33, 0:64], ft[32:33, :],
                                 start=True, stop=True)
                fbq = lns.tile([128, 512], BF16, name="fbq_s", tag="fbq_s")
                fbk = lns.tile([64, 512], BF16, name="fbk_s", tag="fbk_s")
                nc.scalar.copy(fbq[:], fbq_ps[:])
                nc.scalar.copy(fbk[:], fbk_ps[:])
                # rope + scale (DVE)
                t1 = rt.tile([128, 512], BF16, name="t1", tag="t1")
                t2 = rt.tile([128, 512], BF16, name="t2", tag="t2")
                qr0 = rt.tile([128, 512], BF16, name="qr0", tag="qr0")
                qr1 = rt.tile([128, 512], BF16, name="qr1", tag="qr1")
                nc.vector.tensor_mul(t1[:], qsb[0][:, sl], cs[:, slc])
                nc.vector.tensor_mul(t2[:], qsb[1][:, sl], cs[:, sls])
                nc.vector.tensor_add(t1[:], t1[:], t2[:])
                nc.vector.tensor_mul(qr0[:], t1[:], fbq[:])
                u1 = rt.tile([128, 512], BF16, name="u1", tag="u1")
                u2 = rt.tile([128, 512], BF16, name="u2", tag="u2")
                nc.vector.tensor_mul(u1[:], qsb[1][:, sl], cs[:, slc])
                nc.vector.tensor_mul(u2[:], qsb[0][:, sl], cs[:, sls])
                nc.vector.tensor_sub(u1[:], u1[:], u2[:])
                nc.vector.tensor_mul(qr1[:], u1[:], fbq[:])
                k1 = rt.tile([32, 512], BF16, name="k1", tag="k1")
                k2 = rt.tile([32, 512], BF16, name="k2", tag="k2")
                kw0 = rt.tile([32, 512], BF16, name="kw0", tag="kw0")
                kw1 = rt.tile([32, 512], BF16, name="kw1", tag="kw1")
                nc.vector.tensor_mul(k1[:], kvsb[0:32, sl], cs[0:32, slc])
                nc.vector.tensor_mul(k2[:], kb0[:, sl], cs[0:32, sls])
                nc.vector.tensor_add(k1[:], k1[:], k2[:])
                nc.vector.tensor_mul(kw0[:], k1[:], fbk[0:32, :])
                k3 = rt.tile([32, 512], BF16, name="k3", tag="k3")
                k4 = rt.tile([32, 512], BF16, name="k4", tag="k4")
                nc.vector.tensor_mul(k3[:], kb0[:, sl], cs[0:32, slc])
                nc.vector.tensor_mul(k4[:], kvsb[0:32, sl], cs[0:32, sls])
                nc.vector.tensor_sub(k3[:], k3[:], k4[:])
                nc.vector.tensor_mul(kw1[:], k3[:], fbk[0:32, :])
                # reassemble pair-packed q layout (DMA partition moves)
                for h, (rb, slot) in ((0, (0, 0)), (2, (0, 1)),
                                      (1, (64, 0)), (3, (64, 1))):
                    hs = slice(32 * h, 32 * h + 32)
                    bsl = slice(2 * n, 2 * n + 2)
                    nc.sync.dma_start(
                        qp[rb:rb + 32, bsl, slot, :],
                        qr0[hs, :].rearrange("p (b c) -> p b c", b=2))
                    nc.sync.dma_start(
                        qp[rb + 32:rb + 64, bsl, slot, :],
                        qr1[hs, :].rearrange("p (b c) -> p b c", b=2))
                nc.sync.dma_start(kdup[0:32, sl], kw0[:])
                nc.sync.dma_start(kdup[32:64, sl], kw1[:])
                nc.sync.dma_start(kdup[64:96, sl], kw0[:])
                nc.sync.dma_start(kdup[96:128, sl], kw1[:])

            proj_chunk(0)
            for n in range(1, NC):
                proj_chunk(n)
                post_chunk(n - 1)
            post_chunk(NC - 1)

        # ======== phase 2+3: attention blocks with interleaved Wo ========
        with (
            tc.tile_pool(name="ps", bufs=2, space=bass.MemorySpace.PSUM) as ps,
            tc.tile_pool(name="py01", bufs=2, space=bass.MemorySpace.PSUM) as py01,
            tc.tile_pool(name="py23", bufs=2, space=bass.MemorySpace.PSUM) as py23,
            tc.tile_pool(name="pa", bufs=4) as pa,
            tc.tile_pool(name="sst", bufs=2) as sst,
            tc.tile_pool(name="ob", bufs=2) as ob,
        ):
            def emit_wo(b):
                """Output projection for sq block b (yn cols 256b:256b+256)."""
                for t2 in range(2):
                    sti = 2 * b + t2
                    ssl = slice(128 * sti, 128 * (sti + 1))
                    ot = ob.tile([128, D], F16, name="ot", tag="ot")
                    for half in range(2):
                        dsl = slice(512 * half, 512 * (half + 1))
                        pot = ps.tile([128, 512], F32, name="st", tag="st")
                        nc.tensor.matmul(pot[:], ynA[:, 0, ssl], wo[:, 0, dsl],
                                         start=True, stop=False)
                        nc.tensor.matmul(pot[:], ynA[:, 1, ssl], wo[:, 1, dsl],
                                         start=False, stop=True)
                        # drains split DVE/ACT to balance engine load
                        if half == 0:
                            nc.scalar.copy(ot[:, dsl], pot[:])
                        else:
                            nc.vector.tensor_copy(ot[:, dsl], pot[:])
                    nc.sync.dma_start(out_d[ssl, :], ot[:])

            def finish_normalize(st8):
                """PE broadcast + multiplies for a block whose recip is ready."""
                b, yt01, yt23, rbc = st8
                sq = slice(SQB * b, SQB * (b + 1))
                pbb = ps.tile([128, 512], F32, name="st", tag="st")
                nc.tensor.matmul(pbb[0:64, :], onesf16[0:1, 0:64],
                                 rbc[0:1, 0:512],
                                 start=True, stop=True, skip_group_check=True)
                nc.tensor.matmul(pbb[64:128, :], onesf16[0:1, 0:64],
                                 rbc[0:1, 512:1024],
                                 start=True, stop=True, skip_group_check=True)
                nc.vector.tensor_copy(pbs[:], pbb[:])
                # h0 -> yn slot 0 rows 0:64, h2 -> slot 1 rows 0:64 (direct)
                nc.vector.tensor_mul(
                    ynA[0:64, :, sq],
                    yt01[0:64, :].rearrange("p (u c) -> p u c", u=2),
                    pbs[0:64, :].rearrange("p (u c) -> p u c", u=2))
                # h1/h3 -> yn rows 64:128 (via shift DMA)
                sg1 = sst.tile([64, 2, 256], BF16, name="sg1", tag="sg1")
                nc.vector.tensor_mul(
                    sg1[:],
                    yt23[0:64, :].rearrange("p (u c) -> p u c", u=2),
                    pbs[64:128, :].rearrange("p (u c) -> p u c", u=2))
                nc.sync.dma_start(ynA[64:128, :, sq], sg1[:])
                if KDEBUG and b == 0:
                    nc.sync.dma_start(dbg["d_pbs"][:], pbs[:])

            pend = None
            for b in range(NB):
                sq = slice(SQB * b, SQB * (b + 1))
                jmax = 2 * b + 1
                # inject the deferred (previous-block) normalize + Wo early in
                # this block's j-loop so the PE flows straight across the
                # block boundary
                inject_at = min(4, jmax)
                yt01 = py01.tile([65, 512], F32, name="yt01", tag="yt01")
                yt23 = py23.tile([65, 512], F32, name="yt23", tag="yt23")
                for j in range(jmax + 1):
                    if j == inject_at:
                        if pend is not None:
                            finish_normalize(pend)
                        if b >= 2:
                            emit_wo(b - 2)
                    jt = slice(128 * j, 128 * (j + 1))
                    stl = ps.tile([128, 1024], F32, name="st", tag="st")
                    diag = j - 2 * b
                    if diag >= 0:
                        mbsl = slice(512 * diag, 512 * (diag + 1))
                        nc.tensor.matmul(stl[:, 0:512], id128_s[:], mbt[:, mbsl],
                                         start=True, stop=False,
                                         skip_group_check=True)
                        nc.tensor.matmul(stl[:, 512:1024], id128_s[:], mbt[:, mbsl],
                                         start=True, stop=False,
                                         skip_group_check=True)
                    sflag = diag < 0
                    # cols: h0 0:256 | h2 256:512 | h1 512:768 | h3 768:1024
                    nc.tensor.matmul(stl[:, 0:512], kdup[0:64, jt],
                                     qp[0:64, b, :, :], start=sflag, stop=True,
                                     skip_group_check=True)
                    nc.tensor.matmul(stl[:, 512:1024], kdup[64:128, jt],
                                     qp[64:128, b, :, :], start=sflag, stop=True,
                                     skip_group_check=True)
                    pt = pa.tile([128, 1024], BF16, name="pt", tag="pt")
                    nc.scalar.activation(pt[:], stl[:], AF.Exp, bias=e8b[:, :])
                    if KDEBUG and b == 0 and j == 0:
                        nc.sync.dma_start(dbg["d_pt"][:], pt[:])
                    # pv: pair01 = (h0,h2) cols 0:512, pair23 = (h1,h3) 512:1024
                    nc.tensor.matmul(yt01[:], vsb[:, j, 0:65], pt[:, 0:512],
                                     start=(j == 0), stop=(j == jmax),
                                     skip_group_check=True)
                    nc.tensor.matmul(yt23[:], vsb[:, j, 0:65], pt[:, 512:1024],
                                     start=(j == 0), stop=(j == jmax),
                                     skip_group_check=True)
                # normalize, DVE/DMA part; denominators at psum row 64.
                # custom-DVE ops misbehave on HW at partition base 64, so:
                # tracked DVE copy (psum row 64 -> sbuf row 64), DMA shift to
                # partition 0, recip at base 0 (known-good), tracked f16 copy
                # (DVE order covers the untracked recip write).  The PE-side
                # broadcast + multiplies are deferred one block so the PE
                # queue never waits on this chain.
                dcp = sst.tile([65, 1024], F32, name="dcp", tag="dcp")
                nc.vector.tensor_copy(dcp[64:65, 0:512], yt01[64:65, :])
                nc.vector.tensor_copy(dcp[64:65, 512:1024], yt23[64:65, :])
                dlow = sst.tile([1, 1024], F32, name="dlow", tag="dlow")
                nc.sync.dma_start(dlow[0:1, :], dcp[64:65, 0:1024])
                rlow = sst.tile([1, 1024], F32, name="rlow", tag="rlow")
                nc.vector.reciprocal_approx_fast(rlow[0:1, :], dlow[0:1, :])
                rbc = sst.tile([1, 1024], F16, name="rbc", tag="rbc")
                nc.vector.tensor_copy(rbc[0:1, :], rlow[0:1, :])
                if KDEBUG and b == 0:
                    dstage = sst.tile([65, 512], F32, name="dst", tag="dst")
                    dstage2 = sst.tile([65, 512], F32, name="dst2", tag="dst2")
                    nc.vector.tensor_copy(dstage[64:65, :], yt01[64:65, :])
                    nc.vector.tensor_copy(dstage2[64:65, :], yt23[64:65, :])
                    nc.sync.dma_start(dbg["d_den"][0:1, :], dstage[64:65, :])
                    nc.sync.dma_start(dbg["d_den"][1:2, :], dstage2[64:65, :])
                    nc.vector.tensor_copy(dstage[0:64, :], yt01[0:64, :])
                    nc.vector.tensor_copy(dstage2[0:64, :], yt23[0:64, :])
                    nc.sync.dma_start(dbg["d_yt01"][:], dstage[0:64, :])
                    nc.sync.dma_start(dbg["d_yt23"][:], dstage2[0:64, :])
                    nc.sync.dma_start(dbg["d_rbv"][0:1, :], rlow[0:1, 0:512])
                    nc.sync.dma_start(dbg["d_rbv"][1:2, :], rlow[0:1, 512:1024])
                pend = (b, yt01, yt23, rbc)
            finish_normalize(pend)
            emit_wo(NB - 2)
            emit_wo(NB - 1)
            if KDEBUG:
                nc.sync.dma_start(dbg["d_qsb0"][:], qsb[0][:])
                nc.sync.dma_start(dbg["d_qsb1"][:], qsb[1][:])
                nc.sync.dma_start(dbg["d_kvsb"][:], kvsb[:])
                nc.sync.dma_start(dbg["d_kdup"][:], kdup[:])
                nc.sync.dma_start(dbg["d_vsb"][:], vsb[:])
                nc.sync.dma_start(dbg["d_yn0"][:], ynA[:, 0, :])
                nc.sync.dma_start(dbg["d_yn1"][:], ynA[:, 1, :])

    nc.finalize()
    return nc


_NC = None


def _get_nc():
    global _NC
    if _NC is None:
        _NC = _build()
    return _NC


def _perm():
    tops = [h * 64 + i for h in range(HG) for i in range(32)]
    bots = [h * 64 + 32 + i for h in range(HG) for i in range(32)]
    return tops + bots


def build_inmaps(x, Wq, Wk, Wv, Wo, q_gain):
    x = np.asarray(x, dtype=np.float32)
    Wq = np.asarray(Wq, dtype=np.float32)
    Wk = np.asarray(Wk, dtype=np.float32)
    Wv = np.asarray(Wv, dtype=np.float32)
    Wo = np.asarray(Wo, dtype=np.float32)
    q_gain = np.asarray(q_gain, dtype=np.float32)

    perm = _perm()
    xTs = [np.ascontiguousarray(x[dp].T).astype(BF16NP) for dp in range(2)]
    tp_maps = []
    for tp in range(4):
        wq_sel = Wq[tp * E:(tp + 1) * E].T[:, perm]          # [D, 256] permuted
        wq_t = np.ascontiguousarray(
            wq_sel.astype(BF16NP).reshape(NK, 128, E).transpose(1, 0, 2))
        wk_sel = Wk[tp * HD:(tp + 1) * HD].T                  # [D, 64]
        wv_sel = Wv[tp * HD:(tp + 1) * HD].T
        wkv_t = np.concatenate([wk_sel, wv_sel], axis=1).astype(BF16NP)
        wkv_t = np.ascontiguousarray(
            wkv_t.reshape(NK, 128, 128).transpose(1, 0, 2))
        wo_sel = Wo[:, tp * E:(tp + 1) * E].T                 # [256, D]
        wo_t = np.ascontiguousarray(
            wo_sel.astype(BF16NP).reshape(2, 128, D).transpose(1, 0, 2))
        g = q_gain[tp * HG:(tp + 1) * HG].astype(np.float32)
        gsel = np.zeros((4, 128), dtype=BF16NP)
        for h in range(4):
            gsel[h, 32 * h:32 * h + 32] = BF16NP(g[h] / 8.0)
        tp_maps.append({"wq": wq_t, "wkv": wkv_t, "wo": wo_t, "gsel": gsel})
    in_maps = []
    for c in range(8):
        dp, tp = divmod(c, 4)
        m = dict(tp_maps[tp])
        m["xT"] = xTs[dp]
        in_maps.append(m)
    return in_maps


def kernel(x, Wq, Wk, Wv, Wo, q_gain):
    in_maps = build_inmaps(x, Wq, Wk, Wv, Wo, q_gain)
    nc = _get_nc()
    res = run_bass_kernel_spmd(nc, in_maps, core_ids=list(range(8)))
    out = np.zeros((B, S, D), dtype=np.float32)
    for c in range(8):
        out[c // 4] += res.results[c]["out"].astype(np.float32)
    return out
